# revision 3
# baseline (speedup 1.0000x reference)
"""Quantized int8 matmul on 8 TRN2 NeuronCores.

Math: out = ((x - ZP_X) * SCALE_X) @ ((y - ZP_Y) * SCALE_Y)
Implemented as: out = [(x - ZP_X) @ (y - ZP_Y)] * (SCALE_X * SCALE_Y)
The zero-point-shifted int8 values (range ~[-150, 155]) are exactly
representable in bf16, so a bf16 matmul with fp32 PSUM accumulation is
numerically ~identical to the fp32 reference.

Sharding: x row-sharded (M) across 8 cores, y replicated, no collectives.
Per core: x_loc [512, 4096] i8, y [4096, 4096] i8 -> out_loc [512, 4096] f32.
"""

import numpy as np

SCALE_X, ZP_X = 0.0215, -25
SCALE_Y, ZP_Y = 0.0176, 18
M, K, N = 4096, 4096, 4096
N_CORES = 8
P = 128
NBLK = 512  # matmul free dim = one PSUM bank of fp32


def build_nc(m_loc, k, n):
    from contextlib import ExitStack

    import concourse.mybir as mybir
    import concourse.tile as tile
    from concourse import bacc
    from concourse.bass import ts
    from concourse.masks import make_identity

    fp32 = mybir.dt.float32
    bf16 = mybir.dt.bfloat16
    int8 = mybir.dt.int8
    Copy = mybir.ActivationFunctionType.Copy

    MT = m_loc // P  # partition tiles of x rows
    KT = k // P  # contraction tiles
    NB = n // NBLK  # output column blocks

    nc = bacc.Bacc(None, debug=False)
    x = nc.declare_dram_parameter("x", [m_loc, k], int8, isOutput=False)
    y = nc.declare_dram_parameter("y", [k, n], int8, isOutput=False)
    out = nc.declare_dram_parameter("out", [m_loc, n], fp32, isOutput=True)

    with ExitStack() as ctx:
        tc = ctx.enter_context(tile.TileContext(nc))
        const = ctx.enter_context(tc.tile_pool(name="const", bufs=1))
        xi_pool = ctx.enter_context(tc.tile_pool(name="xi", bufs=2))
        xb_pool = ctx.enter_context(tc.tile_pool(name="xb", bufs=2))
        xt_pool = ctx.enter_context(tc.tile_pool(name="xt", bufs=1))
        yi_pool = ctx.enter_context(tc.tile_pool(name="yi", bufs=6))
        yb_pool = ctx.enter_context(tc.tile_pool(name="yb", bufs=6))
        ob_pool = ctx.enter_context(tc.tile_pool(name="ob", bufs=4))
        ps_pool = ctx.enter_context(tc.tile_pool(name="ps", bufs=8, space="PSUM"))

        ident = const.tile([P, P], bf16)
        make_identity(nc, ident)

        # Persistent transposed x: partition = k within tile, free = (kt, m)
        xT = xt_pool.tile([P, KT, m_loc], bf16)

        # Phase 1: load x, i8 -> bf16 with -ZP_X bias, transpose via PE
        for mt in range(MT):
            xi = xi_pool.tile([P, k], int8)
            nc.sync.dma_start(xi[:], x[ts(mt, P), :])
            xb = xb_pool.tile([P, k], bf16)
            nc.vector.tensor_scalar_add(xb[:], xi[:], float(-ZP_X))
            for kt in range(KT):
                pt = ps_pool.tile([P, P], bf16, tag="ps")
                nc.tensor.transpose(pt[:], xb[:, ts(kt, P)], ident[:])
                nc.vector.tensor_copy(xT[:, kt, ts(mt, P)], pt[:])

        # Phase 2: stream y, accumulate in PSUM, scale on evict
        for nb in range(NB):
            psums = [
                ps_pool.tile([P, NBLK], fp32, tag="ps", name=f"acc_{nb}_{i}")
                for i in range(MT)
            ]
            for kt in range(KT):
                yi = yi_pool.tile([P, NBLK], int8)
                nc.sync.dma_start(yi[:], y[ts(kt, P), ts(nb, NBLK)])
                yb = yb_pool.tile([P, NBLK], bf16)
                if kt % 2 == 0:
                    nc.vector.tensor_scalar_add(yb[:], yi[:], float(-ZP_Y))
                else:
                    nc.scalar.activation(yb[:], yi[:], Copy, bias=float(-ZP_Y))
                for mt in range(MT):
                    nc.tensor.matmul(
                        psums[mt][:],
                        xT[:, kt, ts(mt, P)],
                        yb[:],
                        start=(kt == 0),
                        stop=(kt == KT - 1),
                    )
            for mt in range(MT):
                ob = ob_pool.tile([P, NBLK], fp32)
                nc.scalar.activation(
                    ob[:], psums[mt][:], Copy, scale=float(SCALE_X * SCALE_Y)
                )
                nc.sync.dma_start(out[ts(mt, P), ts(nb, NBLK)], ob[:])

    nc.compile()
    return nc


_NC_CACHE = None
LAST_RESULT = None  # BassKernelResults of the most recent run (for profiling)


def kernel(x, y):
    global _NC_CACHE, LAST_RESULT
    from concourse.bass_utils import run_bass_kernel_spmd

    x = np.asarray(x)
    y = np.asarray(y)
    assert x.shape == (M, K) and y.shape == (K, N), (x.shape, y.shape)
    x8 = x.astype(np.int8) if x.dtype != np.int8 else x
    y8 = y.astype(np.int8) if y.dtype != np.int8 else y

    if _NC_CACHE is None:
        _NC_CACHE = build_nc(M // N_CORES, K, N)
    nc = _NC_CACHE

    m_loc = M // N_CORES
    in_maps = [
        {"x": np.ascontiguousarray(x8[i * m_loc : (i + 1) * m_loc]), "y": y8}
        for i in range(N_CORES)
    ]
    res = run_bass_kernel_spmd(nc, in_maps, core_ids=list(range(N_CORES)))
    LAST_RESULT = res
    return np.concatenate(
        [np.asarray(res.results[i]["out"]) for i in range(N_CORES)], axis=0
    )
